# revision 13
# baseline (speedup 1.0000x reference)
"""Trainium2 Bass kernel for nn_Attention_layer_12249246728743.

Structure of the reference computation (after untangling the C-order
reshape): per channel c of 512, the 3136 raster positions split into 49
segments of 64 consecutive positions; each segment attends over a 7x7
shifted window of its OWN channel plane (depthwise local attention):

  scores[c,s,p=(i,j)] = sum_d q[c,64s+d] * k[c, win(64s+d, i, j)]
                        + (sum_d q[c,64s+d]) * bias49[p]
  w = softmax_p(scores);  out[c,64s+d] = sum_p w[c,s,p] * v[c, win(...)]

with q/k/v = 1x1 convs of x (k, v on the zero-padded 62x62 domain).

Sharding: channel-parallel across 8 cores. Core r owns channels
{64h + 8r + t : h in 0..7, t in 0..7} (64 channels), so every attention
segment is core-local: no halo, no collectives. x is replicated; weight
rows are gathered per core on host.

On-device per core: 1x1 convs on the PE array (contraction over 512 input
channels, 4 K-tiles), bias folded into PSUM->SBUF eviction on the scalar
engine; attention on the vector engine with a 128-partition layout
(channel, image-half) and the qsum*bias term algebraically folded into the
qk product: (K_win + bias_p) * Q reduced over d.
"""

import numpy as np

import concourse.bass as bass
import concourse.mybir as mybir
import concourse.tile as tile
from concourse.bass_utils import run_bass_kernel_spmd

F32 = mybir.dt.float32
BF16 = mybir.dt.bfloat16
AX = mybir.AxisListType
OP = mybir.AluOpType
AF = mybir.ActivationFunctionType

N_CORES = 8
C = 512
H = W = 56
HP = WP = 62          # padded spatial
NPOS = H * W          # 3136
NPAD = HP * WP        # 3844
K = 7
NSH = K * K           # 49 shifts
SEG = 64              # positions per attention segment
NSEG = NPOS // SEG    # 49 segments per channel
CH = 64               # channels per core

# image-half split: half0 = out rows 0..31 (28 segs), half1 = rows 32..55 (21 segs)
H0_ROWS, H1_ROWS = 32, 24
H0_POS, H1_POS = H0_ROWS * W, H1_ROWS * W      # 1792, 1344
H0_SEG, H1_SEG = H0_POS // SEG, H1_POS // SEG  # 28, 21
# padded-row ranges needed per half for the 7-row windows
H0_KROWS, H1_KROWS = H0_ROWS + K - 1, H1_ROWS + K - 1   # 38, 30
KW0, KW1 = H0_KROWS * WP, H1_KROWS * WP                 # 2356, 1860
H1_KOFF = 32 * WP                                       # padded row 32 start


def _build_nc():
    nc = bass.Bass()

    xp = nc.declare_dram_parameter("xp", [C, NPAD], F32, isOutput=False)
    wT = nc.declare_dram_parameter("wT", [C, 3 * CH], F32, isOutput=False)
    bqk = nc.declare_dram_parameter("bqk", [2 * CH, 1], F32, isOutput=False)
    bv = nc.declare_dram_parameter("bv", [CH, 1], F32, isOutput=False)
    b49 = nc.declare_dram_parameter("b49", [128, NSH], F32, isOutput=False)
    out_d = nc.declare_dram_parameter("out", [CH, NPOS], F32, isOutput=True)

    NCHUNK = 512
    chunks = [(c0, min(NCHUNK, NPAD - c0)) for c0 in range(0, NPAD, NCHUNK)]

    with tile.TileContext(nc) as tc:
        with (
            tc.tile_pool(name="persist", bufs=1) as pp,
            tc.tile_pool(name="work", bufs=2) as wp,
            tc.tile_pool(name="psum", bufs=2, space="PSUM") as psp,
        ):
            # ---- loads (batched into few DMAs to bound per-inst sem waits) ----
            xt_all = pp.tile([128, 4 * NPAD], F32, tag="xall", name="xall")
            wt_all = pp.tile([128, 4 * 3 * CH], F32, tag="wall", name="wall")
            for s0 in range(0, NPAD, 1024):
                sn = min(1024, NPAD - s0)
                nc.sync.dma_start(
                    xt_all[:].rearrange("p (k n) -> p k n", k=4)[:, :, s0:s0 + sn],
                    xp[:].rearrange("(k p) n -> p k n", p=128)[:, :, s0:s0 + sn])
            nc.sync.dma_start(
                wt_all[:].rearrange("p (k n) -> p k n", k=4),
                wT[:].rearrange("(k p) n -> p k n", p=128))
            xt = [xt_all[:].rearrange("p (k n) -> p k n", k=4)[:, kt, :]
                  for kt in range(4)]
            wt = [wt_all[:].rearrange("p (k n) -> p k n", k=4)[:, kt, :]
                  for kt in range(4)]
            bqk_s = pp.tile([128, 1], F32, tag="bqk", name="bqk")
            bv_s = pp.tile([CH, 1], F32, tag="bv", name="bv")
            b49_s = pp.tile([128, NSH], F32, tag="b49", name="b49")
            nc.sync.dma_start(bqk_s[:], bqk[:])
            nc.sync.dma_start(bv_s[:], bv[:])
            nc.sync.dma_start(b49_s[:], b49[:])

            # ---- conv staging (channel-major, padded domain) ----
            qs = pp.tile([CH, NPAD], BF16, tag="qs", name="qs")
            qs32 = pp.tile([CH, NPAD], F32, tag="qs32", name="qs32")
            ks = pp.tile([CH, NPAD], BF16, tag="ks", name="ks")
            vs = pp.tile([CH, NPAD], BF16, tag="vs", name="vs")

            # PE pre-touch of xall: keeps every real Matmult at <=1 sem wait
            # (walrus S3_LW codegen rejects multi-wait matmuls).
            dmy = psp.tile([1, 1], F32, tag="dmy", name="dmy")
            nc.tensor.matmul(dmy[:], lhsT=xt_all[0:1, 0:1],
                             rhs=xt_all[0:1, 0:1], start=True, stop=True)

            for c0, n in chunks:
                ps_qk = psp.tile([128, NCHUNK], F32, tag="psqk", name="psqk")
                ps_v = psp.tile([CH, NCHUNK], F32, tag="psv", name="psv")
                for kt in range(4):
                    nc.tensor.matmul(
                        ps_qk[:, :n], lhsT=wt[kt][:, 0:128],
                        rhs=xt[kt][:, c0:c0 + n],
                        start=(kt == 0), stop=(kt == 3))
                    nc.tensor.matmul(
                        ps_v[:, :n], lhsT=wt[kt][:, 128:192],
                        rhs=xt[kt][:, c0:c0 + n],
                        start=(kt == 0), stop=(kt == 3))
                sl = slice(c0, c0 + n)
                nc.scalar.activation(qs[0:CH, sl], ps_qk[0:CH, :n], AF.Identity,
                                     bias=bqk_s[0:CH, :])
                nc.scalar.activation(qs32[0:CH, sl], ps_qk[0:CH, :n],
                                     AF.Identity, bias=bqk_s[0:CH, :])
                nc.scalar.activation(ks[0:CH, sl], ps_qk[CH:128, :n], AF.Identity,
                                     bias=bqk_s[CH:128, :])
                nc.scalar.activation(vs[0:CH, sl], ps_v[0:CH, :n], AF.Identity,
                                     bias=bv_s[:])

            # ---- remap to 128-partition attention layout (bf16) ----
            qa = pp.tile([128, H0_POS], BF16, tag="qa", name="qa")
            ka = pp.tile([128, KW0], BF16, tag="ka", name="ka")
            va = pp.tile([128, KW0], BF16, tag="va", name="va")
            nc.vector.memset(qa[CH:128, H1_POS:H0_POS], 0.0)
            nc.vector.memset(ka[CH:128, KW1:KW0], 0.0)
            nc.vector.memset(va[CH:128, KW1:KW0], 0.0)

            qs3 = qs[:].rearrange("a (r c) -> a r c", c=WP)
            # central 56x56 of the padded q plane
            nc.sync.dma_start(
                qa[0:CH, :].rearrange("a (x y) -> a x y", y=W),
                qs3[:, 3:3 + H0_ROWS, 3:3 + W])
            nc.sync.dma_start(
                qa[CH:128, 0:H1_POS].rearrange("a (x y) -> a x y", y=W),
                qs3[:, 3 + H0_ROWS:3 + H, 3:3 + W])
            nc.sync.dma_start(ka[0:CH, :], ks[:, 0:KW0])
            nc.sync.dma_start(ka[CH:128, 0:KW1], ks[:, H1_KOFF:NPAD])
            nc.sync.dma_start(va[0:CH, :], vs[:, 0:KW0])
            nc.sync.dma_start(va[CH:128, 0:KW1], vs[:, H1_KOFF:NPAD])

            # odd-element-shifted copies keep every window 4B-aligned so
            # bf16 tensor_tensor stays in 2x mode for odd j shifts
            kao = pp.tile([128, KW0], BF16, tag="kao", name="kao")
            vao = pp.tile([128, KW0], BF16, tag="vao", name="vao")
            nc.scalar.copy(kao[:, 0:KW0 - 1], ka[:, 1:KW0])
            nc.scalar.copy(vao[:, 0:KW0 - 1], va[:, 1:KW0])

            qa32 = pp.tile([128, H0_POS], F32, tag="qa32", name="qa32")
            nc.vector.memset(qa32[CH:128, H1_POS:H0_POS], 0.0)
            qs323 = qs32[:].rearrange("a (r c) -> a r c", c=WP)
            nc.sync.dma_start(
                qa32[0:CH, :].rearrange("a (x y) -> a x y", y=W),
                qs323[:, 3:3 + H0_ROWS, 3:3 + W])
            nc.sync.dma_start(
                qa32[CH:128, 0:H1_POS].rearrange("a (x y) -> a x y", y=W),
                qs323[:, 3 + H0_ROWS:3 + H, 3:3 + W])

            qa3 = qa[:].rearrange("a (x y) -> a x y", y=W)        # [128,32,56]

            def win(t, i, j):
                src_t, jj = (t[0], j) if j % 2 == 0 else (t[1], j - 1)
                t3 = src_t[:].rearrange("a (r c) -> a r c", c=WP)
                return t3[:, i:i + H0_ROWS, jj:jj + W]

            # ---- qk: scores[part, seg, p] (bias added afterwards) ----
            S = pp.tile([128, H0_SEG * NSH], F32, tag="S", name="S")
            S3 = S[:].rearrange("a (s q) -> a s q", q=NSH)
            for p in range(NSH):
                i, j = divmod(p, K)
                prod = wp.tile([128, H0_POS], BF16, tag="prod", name="prod",
                               bufs=3)
                eng = nc.gpsimd if p % 2 == 1 else nc.vector
                eng.tensor_tensor(
                    out=prod[:].rearrange("a (x y) -> a x y", y=W),
                    in0=win((ka, kao), i, j), in1=qa3, op=OP.mult)
                nc.vector.tensor_reduce(
                    out=S3[:, :, p:p + 1],
                    in_=prod[:].rearrange("a (s d) -> a s d", d=SEG),
                    axis=AX.X, op=OP.add)

            # ---- + qsum * bias49 (rank-1), then exp / denominators ----
            qsum = pp.tile([128, H0_SEG], F32, tag="qsum", name="qsum")
            nc.vector.tensor_reduce(
                out=qsum[:],
                in_=qa32[:].rearrange("a (s d) -> a s d", d=SEG),
                axis=AX.X, op=OP.add)
            tb = pp.tile([128, H0_SEG * NSH], F32, tag="tb", name="tb")
            tb3 = tb[:].rearrange("a (s q) -> a s q", q=NSH)
            nc.vector.tensor_tensor(
                out=tb3,
                in0=qsum[:].rearrange("a (s o) -> a s o", o=1).broadcast_to(
                    (128, H0_SEG, NSH)),
                in1=b49_s[:].rearrange("a (o q) -> a o q", o=1).broadcast_to(
                    (128, H0_SEG, NSH)),
                op=OP.mult)
            sb = pp.tile([128, H0_SEG * NSH], F32, tag="sb", name="sb")
            nc.vector.tensor_tensor(out=sb[:], in0=S[:], in1=tb[:], op=OP.add)
            # the rank-1 bias term reaches +-100: must subtract the max
            # before exp or fp32 overflows
            sb3 = sb[:].rearrange("a (s q) -> a s q", q=NSH)
            mx = pp.tile([128, H0_SEG], F32, tag="mx", name="mx")
            nc.vector.tensor_reduce(out=mx[:], in_=sb3, axis=AX.X, op=OP.max)
            nc.vector.tensor_tensor(
                out=sb3, in0=sb3,
                in1=mx[:].rearrange("a (s o) -> a s o", o=1).broadcast_to(
                    (128, H0_SEG, NSH)),
                op=OP.subtract)
            E = pp.tile([128, H0_SEG * NSH], F32, tag="E", name="E")
            nc.scalar.activation(E[:], sb[:], AF.Exp)
            E3 = E[:].rearrange("a (s q) -> a s q", q=NSH)
            den = pp.tile([128, H0_SEG], F32, tag="den", name="den")
            nc.vector.tensor_reduce(out=den[:], in_=E3, axis=AX.X, op=OP.add)
            rcp = pp.tile([128, H0_SEG], F32, tag="rcp", name="rcp")
            nc.vector.reciprocal(rcp[:], den[:])

            # ---- av: acc[part, pos] = sum_p w_p * V_win_p (bf16 chain) ----
            accA = pp.tile([128, H0_POS], BF16, tag="accA", name="accA")
            accB = pp.tile([128, H0_POS], BF16, tag="accB", name="accB")
            for p in range(NSH):
                i, j = divmod(p, K)
                wexp = wp.tile([128, H0_POS], BF16, tag="wexp", name="wexp",
                               bufs=3)
                nc.scalar.copy(
                    out=wexp[:].rearrange("a (s d) -> a s d", d=SEG),
                    in_=E3[:, :, p:p + 1].broadcast_to((128, H0_SEG, SEG)))
                wx = wexp[:].rearrange("a (x y) -> a x y", y=W)
                if p == 0:
                    nc.vector.tensor_tensor(
                        out=accA[:].rearrange("a (x y) -> a x y", y=W),
                        in0=wx, in1=win((va, vao), i, j), op=OP.mult)
                else:
                    tmp = wp.tile([128, H0_POS], BF16, tag="avt", name="avt",
                                  bufs=3)
                    eng = nc.gpsimd if p % 4 == 2 else nc.vector
                    eng.tensor_tensor(
                        out=tmp[:].rearrange("a (x y) -> a x y", y=W),
                        in0=wx, in1=win((va, vao), i, j), op=OP.mult)
                    src_t, dst = (accA, accB) if p % 2 == 1 else (accB, accA)
                    nc.vector.tensor_tensor(
                        out=dst[:], in0=src_t[:], in1=tmp[:], op=OP.add)
            acc = accA if (NSH - 1) % 2 == 0 else accB

            # ---- normalize (fp32 out) and store ----
            fin = pp.tile([128, H0_POS], F32, tag="fin", name="fin")
            rcpb = rcp[:].rearrange("a (s o) -> a s o", o=1).broadcast_to(
                (128, H0_SEG, SEG))
            nc.vector.tensor_tensor(
                out=fin[:].rearrange("a (s d) -> a s d", d=SEG),
                in0=acc[:].rearrange("a (s d) -> a s d", d=SEG),
                in1=rcpb, op=OP.mult)
            nc.sync.dma_start(out_d[:, 0:H0_POS], fin[0:CH, :])
            nc.sync.dma_start(out_d[:, H0_POS:NPOS], fin[CH:128, 0:H1_POS])
    return nc


import json


def _legalize_waits(bir_bytes):
    """Walrus codegen rejects >1 semaphore wait per instruction; hoist the
    extras onto NoOps (same engine, immediately before) so every
    instruction carries at most one wait."""
    bir = json.loads(bir_bytes)
    ctr = [0]

    def fix_block(instructions):
        out = []
        for ins in instructions:
            si = ins.get("sync_info")
            if si:
                w = si.get("on_wait") or []
                if len(w) > 1:
                    for extra in w[:-1]:
                        ctr[0] += 1
                        out.append({
                            "debug": ins.get("debug", 0),
                            "engine": ins["engine"],
                            "ins": [], "outs": [],
                            "name": f"I-lw{ctr[0]}",
                            "opcode": "NoOp",
                            "sync_info": {"on_wait": [extra],
                                          "on_update": []},
                        })
                    si["on_wait"] = [w[-1]]
            out.append(ins)
        instructions[:] = out

    def walk(o):
        if isinstance(o, dict):
            if "instructions" in o:
                fix_block(o["instructions"])
            for v in o.values():
                walk(v)
        elif isinstance(o, list):
            for v in o:
                walk(v)

    walk(bir)
    return json.dumps(bir).encode()


_NC_CACHE = {}


def kernel(x, q_w, q_b, k_w, k_b, v_w, v_b, h_pos, w_pos):
    x = np.asarray(x, np.float32)
    xp = np.pad(x[0], ((0, 0), (3, 3), (3, 3))).reshape(C, NPAD)
    bias49 = (np.asarray(h_pos, np.float32).sum(0)
              + np.asarray(w_pos, np.float32).sum(0)).reshape(NSH)
    b49bc = np.ascontiguousarray(np.tile(bias49[None, :], (128, 1)))

    in_maps = []
    chan_lists = []
    for r in range(N_CORES):
        chans = np.array([64 * h + 8 * r + t for h in range(8)
                          for t in range(8)])
        chan_lists.append(chans)
        wq = np.asarray(q_w, np.float32)[chans, :]
        wk = np.asarray(k_w, np.float32)[chans, :]
        wv = np.asarray(v_w, np.float32)[chans, :]
        wT = np.ascontiguousarray(
            np.concatenate([wq.T, wk.T, wv.T], axis=1))
        bqk = np.concatenate([np.asarray(q_b, np.float32)[chans],
                              np.asarray(k_b, np.float32)[chans]])
        in_maps.append({
            "xp": xp,
            "wT": wT,
            "bqk": np.ascontiguousarray(bqk[:, None]),
            "bv": np.ascontiguousarray(
                np.asarray(v_b, np.float32)[chans][:, None]),
            "b49": b49bc,
        })

    if "nc" not in _NC_CACHE:
        nc = _build_nc()
        legal = _legalize_waits(nc.to_json_bytes())
        nc.to_json_bytes = lambda: legal
        _NC_CACHE["nc"] = nc
    res = run_bass_kernel_spmd(_NC_CACHE["nc"], in_maps,
                               list(range(N_CORES)))
    _NC_CACHE["last_results"] = res

    out = np.empty((C, NPOS), np.float32)
    for r in range(N_CORES):
        out[chan_lists[r], :] = np.asarray(res.results[r]["out"])
    return out.reshape(1, C, H, W)


if __name__ == "__main__":
    _build_nc()
    print("build OK")


# revision 15
# speedup vs baseline: 1.0530x; 1.0530x over previous
"""Trainium2 Bass kernel for nn_Attention_layer_12249246728743.

Structure of the reference computation (after untangling the C-order
reshape): per channel c of 512, the 3136 raster positions split into 49
segments of 64 consecutive positions; each segment attends over a 7x7
shifted window of its OWN channel plane (depthwise local attention):

  scores[c,s,p=(i,j)] = sum_d q[c,64s+d] * k[c, win(64s+d, i, j)]
                        + (sum_d q[c,64s+d]) * bias49[p]
  w = softmax_p(scores);  out[c,64s+d] = sum_p w[c,s,p] * v[c, win(...)]

with q/k/v = 1x1 convs of x (k, v on the zero-padded 62x62 domain).

Sharding: channel-parallel across 8 cores. Core r owns channels
{64h + 8r + t : h in 0..7, t in 0..7} (64 channels), so every attention
segment is core-local: no halo, no collectives. x is replicated; weight
rows are gathered per core on host.

On-device per core: 1x1 convs on the PE array (contraction over 512 input
channels, 4 K-tiles), bias folded into PSUM->SBUF eviction on the scalar
engine; attention on the vector engine with a 128-partition layout
(channel, image-half) and the qsum*bias term algebraically folded into the
qk product: (K_win + bias_p) * Q reduced over d.
"""

import numpy as np

import concourse.bass as bass
import concourse.mybir as mybir
import concourse.tile as tile
from concourse.bass_utils import run_bass_kernel_spmd

F32 = mybir.dt.float32
BF16 = mybir.dt.bfloat16
AX = mybir.AxisListType
OP = mybir.AluOpType
AF = mybir.ActivationFunctionType

N_CORES = 8
C = 512
H = W = 56
HP = WP = 62          # padded spatial
NPOS = H * W          # 3136
NPAD = HP * WP        # 3844
K = 7
NSH = K * K           # 49 shifts
SEG = 64              # positions per attention segment
NSEG = NPOS // SEG    # 49 segments per channel
CH = 64               # channels per core

# image-half split: half0 = out rows 0..31 (28 segs), half1 = rows 32..55 (21 segs)
H0_ROWS, H1_ROWS = 32, 24
H0_POS, H1_POS = H0_ROWS * W, H1_ROWS * W      # 1792, 1344
H0_SEG, H1_SEG = H0_POS // SEG, H1_POS // SEG  # 28, 21
# padded-row ranges needed per half for the 7-row windows
H0_KROWS, H1_KROWS = H0_ROWS + K - 1, H1_ROWS + K - 1   # 38, 30
KW0, KW1 = H0_KROWS * WP, H1_KROWS * WP                 # 2356, 1860
H1_KOFF = 32 * WP                                       # padded row 32 start


def _build_nc():
    nc = bass.Bass()

    xp = nc.declare_dram_parameter("xp", [C, NPAD], F32, isOutput=False)
    wT = nc.declare_dram_parameter("wT", [C, 3 * CH], F32, isOutput=False)
    bqk = nc.declare_dram_parameter("bqk", [2 * CH, 1], F32, isOutput=False)
    bv = nc.declare_dram_parameter("bv", [CH, 1], F32, isOutput=False)
    b49 = nc.declare_dram_parameter("b49", [128, NSH], F32, isOutput=False)
    out_d = nc.declare_dram_parameter("out", [CH, NPOS], F32, isOutput=True)

    NCHUNK = 512
    chunks = [(c0, min(NCHUNK, NPAD - c0)) for c0 in range(0, NPAD, NCHUNK)]

    with tile.TileContext(nc) as tc:
        with (
            tc.tile_pool(name="persist", bufs=1) as pp,
            tc.tile_pool(name="work", bufs=2) as wp,
            tc.tile_pool(name="psum", bufs=2, space="PSUM") as psp,
        ):
            # ---- loads (batched into few DMAs to bound per-inst sem waits) ----
            xt_all = pp.tile([128, 4 * NPAD], F32, tag="xall", name="xall")
            wt_all = pp.tile([128, 4 * 3 * CH], F32, tag="wall", name="wall")
            for s0 in range(0, NPAD, 1024):
                sn = min(1024, NPAD - s0)
                nc.sync.dma_start(
                    xt_all[:].rearrange("p (k n) -> p k n", k=4)[:, :, s0:s0 + sn],
                    xp[:].rearrange("(k p) n -> p k n", p=128)[:, :, s0:s0 + sn])
            nc.sync.dma_start(
                wt_all[:].rearrange("p (k n) -> p k n", k=4),
                wT[:].rearrange("(k p) n -> p k n", p=128))
            xt = [xt_all[:].rearrange("p (k n) -> p k n", k=4)[:, kt, :]
                  for kt in range(4)]
            wt = [wt_all[:].rearrange("p (k n) -> p k n", k=4)[:, kt, :]
                  for kt in range(4)]
            bqk_s = pp.tile([128, 1], F32, tag="bqk", name="bqk")
            bv_s = pp.tile([CH, 1], F32, tag="bv", name="bv")
            b49_s = pp.tile([128, NSH], F32, tag="b49", name="b49")
            nc.sync.dma_start(bqk_s[:], bqk[:])
            nc.sync.dma_start(bv_s[:], bv[:])
            nc.sync.dma_start(b49_s[:], b49[:])

            # ---- conv staging (channel-major, padded domain) ----
            qs = pp.tile([CH, NPAD], BF16, tag="qs", name="qs")
            qs32 = pp.tile([CH, NPAD], F32, tag="qs32", name="qs32")
            ks = pp.tile([CH, NPAD], BF16, tag="ks", name="ks")
            vs = pp.tile([CH, NPAD], BF16, tag="vs", name="vs")

            # PE pre-touch of xall: keeps every real Matmult at <=1 sem wait
            # (walrus S3_LW codegen rejects multi-wait matmuls).
            dmy = psp.tile([1, 1], F32, tag="dmy", name="dmy")
            nc.tensor.matmul(dmy[:], lhsT=xt_all[0:1, 0:1],
                             rhs=xt_all[0:1, 0:1], start=True, stop=True)

            for c0, n in chunks:
                ps_qk = psp.tile([128, NCHUNK], F32, tag="psqk", name="psqk")
                ps_v = psp.tile([CH, NCHUNK], F32, tag="psv", name="psv")
                for kt in range(4):
                    nc.tensor.matmul(
                        ps_qk[:, :n], lhsT=wt[kt][:, 0:128],
                        rhs=xt[kt][:, c0:c0 + n],
                        start=(kt == 0), stop=(kt == 3))
                    nc.tensor.matmul(
                        ps_v[:, :n], lhsT=wt[kt][:, 128:192],
                        rhs=xt[kt][:, c0:c0 + n],
                        start=(kt == 0), stop=(kt == 3))
                sl = slice(c0, c0 + n)
                nc.scalar.activation(qs[0:CH, sl], ps_qk[0:CH, :n], AF.Identity,
                                     bias=bqk_s[0:CH, :])
                nc.scalar.activation(qs32[0:CH, sl], ps_qk[0:CH, :n],
                                     AF.Identity, bias=bqk_s[0:CH, :])
                nc.scalar.activation(ks[0:CH, sl], ps_qk[CH:128, :n], AF.Identity,
                                     bias=bqk_s[CH:128, :])
                nc.scalar.activation(vs[0:CH, sl], ps_v[0:CH, :n], AF.Identity,
                                     bias=bv_s[:])

            # ---- remap to 128-partition attention layout (bf16) ----
            qa = pp.tile([128, H0_POS], BF16, tag="qa", name="qa")
            ka = pp.tile([128, KW0], BF16, tag="ka", name="ka")
            va = pp.tile([128, KW0], BF16, tag="va", name="va")
            nc.vector.memset(qa[CH:128, H1_POS:H0_POS], 0.0)
            nc.vector.memset(ka[CH:128, KW1:KW0], 0.0)
            nc.vector.memset(va[CH:128, KW1:KW0], 0.0)

            qs3 = qs[:].rearrange("a (r c) -> a r c", c=WP)
            # central 56x56 of the padded q plane
            nc.sync.dma_start(
                qa[0:CH, :].rearrange("a (x y) -> a x y", y=W),
                qs3[:, 3:3 + H0_ROWS, 3:3 + W])
            nc.sync.dma_start(
                qa[CH:128, 0:H1_POS].rearrange("a (x y) -> a x y", y=W),
                qs3[:, 3 + H0_ROWS:3 + H, 3:3 + W])
            nc.sync.dma_start(ka[0:CH, :], ks[:, 0:KW0])
            nc.sync.dma_start(ka[CH:128, 0:KW1], ks[:, H1_KOFF:NPAD])
            nc.sync.dma_start(va[0:CH, :], vs[:, 0:KW0])
            nc.sync.dma_start(va[CH:128, 0:KW1], vs[:, H1_KOFF:NPAD])

            # odd-element-shifted copies keep every window 4B-aligned so
            # bf16 tensor_tensor stays in 2x mode for odd j shifts
            kao = pp.tile([128, KW0], BF16, tag="kao", name="kao")
            vao = pp.tile([128, KW0], BF16, tag="vao", name="vao")
            nc.scalar.copy(kao[:, 0:KW0 - 1], ka[:, 1:KW0])
            nc.scalar.copy(vao[:, 0:KW0 - 1], va[:, 1:KW0])

            qa32 = pp.tile([128, H0_POS], F32, tag="qa32", name="qa32")
            nc.vector.memset(qa32[CH:128, H1_POS:H0_POS], 0.0)
            qs323 = qs32[:].rearrange("a (r c) -> a r c", c=WP)
            nc.sync.dma_start(
                qa32[0:CH, :].rearrange("a (x y) -> a x y", y=W),
                qs323[:, 3:3 + H0_ROWS, 3:3 + W])
            nc.sync.dma_start(
                qa32[CH:128, 0:H1_POS].rearrange("a (x y) -> a x y", y=W),
                qs323[:, 3 + H0_ROWS:3 + H, 3:3 + W])

            qa3 = qa[:].rearrange("a (x y) -> a x y", y=W)        # [128,32,56]

            def win(t, i, j):
                src_t, jj = (t[0], j) if j % 2 == 0 else (t[1], j - 1)
                t3 = src_t[:].rearrange("a (r c) -> a r c", c=WP)
                return t3[:, i:i + H0_ROWS, jj:jj + W]

            # ---- qk: scores[part, seg, p] (bias added afterwards) ----
            S = pp.tile([128, H0_SEG * NSH], F32, tag="S", name="S")
            S3 = S[:].rearrange("a (s q) -> a s q", q=NSH)
            for p in range(NSH):
                i, j = divmod(p, K)
                prod = wp.tile([128, H0_POS], BF16, tag="prod", name="prod",
                               bufs=2)
                eng = nc.gpsimd if p % 2 == 1 else nc.vector
                eng.tensor_tensor(
                    out=prod[:].rearrange("a (x y) -> a x y", y=W),
                    in0=win((ka, kao), i, j), in1=qa3, op=OP.mult)
                nc.vector.tensor_reduce(
                    out=S3[:, :, p:p + 1],
                    in_=prod[:].rearrange("a (s d) -> a s d", d=SEG),
                    axis=AX.X, op=OP.add)

            # ---- + qsum * bias49 (rank-1), then exp / denominators ----
            qsum = pp.tile([128, H0_SEG], F32, tag="qsum", name="qsum")
            nc.vector.tensor_reduce(
                out=qsum[:],
                in_=qa32[:].rearrange("a (s d) -> a s d", d=SEG),
                axis=AX.X, op=OP.add)
            tb = pp.tile([128, H0_SEG * NSH], F32, tag="tb", name="tb")
            tb3 = tb[:].rearrange("a (s q) -> a s q", q=NSH)
            nc.vector.tensor_tensor(
                out=tb3,
                in0=qsum[:].rearrange("a (s o) -> a s o", o=1).broadcast_to(
                    (128, H0_SEG, NSH)),
                in1=b49_s[:].rearrange("a (o q) -> a o q", o=1).broadcast_to(
                    (128, H0_SEG, NSH)),
                op=OP.mult)
            sb = pp.tile([128, H0_SEG * NSH], F32, tag="sb", name="sb")
            nc.vector.tensor_tensor(out=sb[:], in0=S[:], in1=tb[:], op=OP.add)
            # the rank-1 bias term reaches +-100: must subtract the max
            # before exp or fp32 overflows
            sb3 = sb[:].rearrange("a (s q) -> a s q", q=NSH)
            mx = pp.tile([128, H0_SEG], F32, tag="mx", name="mx")
            nc.vector.tensor_reduce(out=mx[:], in_=sb3, axis=AX.X, op=OP.max)
            nc.vector.tensor_tensor(
                out=sb3, in0=sb3,
                in1=mx[:].rearrange("a (s o) -> a s o", o=1).broadcast_to(
                    (128, H0_SEG, NSH)),
                op=OP.subtract)
            E = pp.tile([128, H0_SEG * NSH], F32, tag="E", name="E")
            nc.scalar.activation(E[:], sb[:], AF.Exp)
            E3 = E[:].rearrange("a (s q) -> a s q", q=NSH)
            den = pp.tile([128, H0_SEG], F32, tag="den", name="den")
            nc.vector.tensor_reduce(out=den[:], in_=E3, axis=AX.X, op=OP.add)
            rcp = pp.tile([128, H0_SEG], F32, tag="rcp", name="rcp")
            nc.vector.reciprocal(rcp[:], den[:])

            # ---- av: acc[part, pos] = sum_p w_p * V_win_p (bf16 chain) ----
            # two independent accumulator chains: DVE owns 37 shifts,
            # GPSIMD owns 12 (p%4==2) end-to-end (mul+add), combined once
            accA = pp.tile([128, H0_POS], BF16, tag="accA", name="accA")
            accB = pp.tile([128, H0_POS], BF16, tag="accB", name="accB")
            accPA = pp.tile([128, H0_POS], BF16, tag="accPA", name="accPA")
            accPB = pp.tile([128, H0_POS], BF16, tag="accPB", name="accPB")
            dve_n = pool_n = 0
            for p in range(NSH):
                i, j = divmod(p, K)
                on_pool = (p % 4 == 2)
                eng = nc.gpsimd if on_pool else nc.vector
                wexp = wp.tile([128, H0_POS], BF16, tag="wexp", name="wexp",
                               bufs=3)
                nc.scalar.copy(
                    out=wexp[:].rearrange("a (s d) -> a s d", d=SEG),
                    in_=E3[:, :, p:p + 1].broadcast_to((128, H0_SEG, SEG)))
                wx = wexp[:].rearrange("a (x y) -> a x y", y=W)
                vwin = win((va, vao), i, j)
                if on_pool:
                    first, pair = pool_n == 0, (accPA, accPB)
                    pool_n += 1
                    k_n = pool_n
                else:
                    first, pair = dve_n == 0, (accA, accB)
                    dve_n += 1
                    k_n = dve_n
                if first:
                    eng.tensor_tensor(
                        out=pair[0][:].rearrange("a (x y) -> a x y", y=W),
                        in0=wx, in1=vwin, op=OP.mult)
                else:
                    tag = "avtP" if on_pool else "avt"
                    tmp = wp.tile([128, H0_POS], BF16, tag=tag, name=tag,
                                  bufs=2)
                    eng.tensor_tensor(
                        out=tmp[:].rearrange("a (x y) -> a x y", y=W),
                        in0=wx, in1=vwin, op=OP.mult)
                    src_t, dst = pair if k_n % 2 == 0 else (pair[1], pair[0])
                    eng.tensor_tensor(
                        out=dst[:], in0=src_t[:], in1=tmp[:], op=OP.add)
            accD = accA if dve_n % 2 == 1 else accB
            accP = accPA if pool_n % 2 == 1 else accPB
            acc = accB if dve_n % 2 == 1 else accA
            nc.vector.tensor_tensor(out=acc[:], in0=accD[:], in1=accP[:],
                                    op=OP.add)

            # ---- normalize (fp32 out) and store ----
            fin = pp.tile([128, H0_POS], F32, tag="fin", name="fin")
            rcpb = rcp[:].rearrange("a (s o) -> a s o", o=1).broadcast_to(
                (128, H0_SEG, SEG))
            nc.vector.tensor_tensor(
                out=fin[:].rearrange("a (s d) -> a s d", d=SEG),
                in0=acc[:].rearrange("a (s d) -> a s d", d=SEG),
                in1=rcpb, op=OP.mult)
            nc.sync.dma_start(out_d[:, 0:H0_POS], fin[0:CH, :])
            nc.sync.dma_start(out_d[:, H0_POS:NPOS], fin[CH:128, 0:H1_POS])
    return nc


import json


def _legalize_waits(bir_bytes):
    """Walrus codegen rejects >1 semaphore wait per instruction; hoist the
    extras onto NoOps (same engine, immediately before) so every
    instruction carries at most one wait."""
    bir = json.loads(bir_bytes)
    ctr = [0]

    def fix_block(instructions):
        out = []
        for ins in instructions:
            si = ins.get("sync_info")
            if si:
                w = si.get("on_wait") or []
                if len(w) > 1:
                    for extra in w[:-1]:
                        ctr[0] += 1
                        out.append({
                            "debug": ins.get("debug", 0),
                            "engine": ins["engine"],
                            "ins": [], "outs": [],
                            "name": f"I-lw{ctr[0]}",
                            "opcode": "NoOp",
                            "sync_info": {"on_wait": [extra],
                                          "on_update": []},
                        })
                    si["on_wait"] = [w[-1]]
            out.append(ins)
        instructions[:] = out

    def walk(o):
        if isinstance(o, dict):
            if "instructions" in o:
                fix_block(o["instructions"])
            for v in o.values():
                walk(v)
        elif isinstance(o, list):
            for v in o:
                walk(v)

    walk(bir)
    return json.dumps(bir).encode()


_NC_CACHE = {}


def kernel(x, q_w, q_b, k_w, k_b, v_w, v_b, h_pos, w_pos):
    x = np.asarray(x, np.float32)
    xp = np.pad(x[0], ((0, 0), (3, 3), (3, 3))).reshape(C, NPAD)
    bias49 = (np.asarray(h_pos, np.float32).sum(0)
              + np.asarray(w_pos, np.float32).sum(0)).reshape(NSH)
    b49bc = np.ascontiguousarray(np.tile(bias49[None, :], (128, 1)))

    in_maps = []
    chan_lists = []
    for r in range(N_CORES):
        chans = np.array([64 * h + 8 * r + t for h in range(8)
                          for t in range(8)])
        chan_lists.append(chans)
        wq = np.asarray(q_w, np.float32)[chans, :]
        wk = np.asarray(k_w, np.float32)[chans, :]
        wv = np.asarray(v_w, np.float32)[chans, :]
        wT = np.ascontiguousarray(
            np.concatenate([wq.T, wk.T, wv.T], axis=1))
        bqk = np.concatenate([np.asarray(q_b, np.float32)[chans],
                              np.asarray(k_b, np.float32)[chans]])
        in_maps.append({
            "xp": xp,
            "wT": wT,
            "bqk": np.ascontiguousarray(bqk[:, None]),
            "bv": np.ascontiguousarray(
                np.asarray(v_b, np.float32)[chans][:, None]),
            "b49": b49bc,
        })

    if "nc" not in _NC_CACHE:
        nc = _build_nc()
        legal = _legalize_waits(nc.to_json_bytes())
        nc.to_json_bytes = lambda: legal
        _NC_CACHE["nc"] = nc
    res = run_bass_kernel_spmd(_NC_CACHE["nc"], in_maps,
                               list(range(N_CORES)))
    _NC_CACHE["last_results"] = res

    out = np.empty((C, NPOS), np.float32)
    for r in range(N_CORES):
        out[chan_lists[r], :] = np.asarray(res.results[r]["out"])
    return out.reshape(1, C, H, W)


if __name__ == "__main__":
    _build_nc()
    print("build OK")


# revision 16
# speedup vs baseline: 1.0951x; 1.0401x over previous
"""Trainium2 Bass kernel for nn_Attention_layer_12249246728743.

Structure of the reference computation (after untangling the C-order
reshape): per channel c of 512, the 3136 raster positions split into 49
segments of 64 consecutive positions; each segment attends over a 7x7
shifted window of its OWN channel plane (depthwise local attention):

  scores[c,s,p=(i,j)] = sum_d q[c,64s+d] * k[c, win(64s+d, i, j)]
                        + (sum_d q[c,64s+d]) * bias49[p]
  w = softmax_p(scores);  out[c,64s+d] = sum_p w[c,s,p] * v[c, win(...)]

with q/k/v = 1x1 convs of x (k, v on the zero-padded 62x62 domain).

Sharding: channel-parallel across 8 cores. Core r owns channels
{64h + 8r + t : h in 0..7, t in 0..7} (64 channels), so every attention
segment is core-local: no halo, no collectives. x is replicated; weight
rows are gathered per core on host.

On-device per core: 1x1 convs on the PE array (contraction over 512 input
channels, 4 K-tiles), bias folded into PSUM->SBUF eviction on the scalar
engine; attention on the vector engine with a 128-partition layout
(channel, image-half) and the qsum*bias term algebraically folded into the
qk product: (K_win + bias_p) * Q reduced over d.
"""

import numpy as np

import concourse.bass as bass
import concourse.mybir as mybir
import concourse.tile as tile
from concourse.bass_utils import run_bass_kernel_spmd

F32 = mybir.dt.float32
BF16 = mybir.dt.bfloat16
AX = mybir.AxisListType
OP = mybir.AluOpType
AF = mybir.ActivationFunctionType

N_CORES = 8
C = 512
H = W = 56
HP = WP = 62          # padded spatial
NPOS = H * W          # 3136
NPAD = HP * WP        # 3844
K = 7
NSH = K * K           # 49 shifts
SEG = 64              # positions per attention segment
NSEG = NPOS // SEG    # 49 segments per channel
CH = 64               # channels per core

# image-half split: half0 = out rows 0..31 (28 segs), half1 = rows 32..55 (21 segs)
H0_ROWS, H1_ROWS = 32, 24
H0_POS, H1_POS = H0_ROWS * W, H1_ROWS * W      # 1792, 1344
H0_SEG, H1_SEG = H0_POS // SEG, H1_POS // SEG  # 28, 21
# padded-row ranges needed per half for the 7-row windows
H0_KROWS, H1_KROWS = H0_ROWS + K - 1, H1_ROWS + K - 1   # 38, 30
KW0, KW1 = H0_KROWS * WP, H1_KROWS * WP                 # 2356, 1860
H1_KOFF = 32 * WP                                       # padded row 32 start


def _build_nc():
    nc = bass.Bass()

    xp = nc.declare_dram_parameter("xp", [C, NPAD], F32, isOutput=False)
    wT = nc.declare_dram_parameter("wT", [C, 3 * CH], F32, isOutput=False)
    bqk = nc.declare_dram_parameter("bqk", [2 * CH, 1], F32, isOutput=False)
    bv = nc.declare_dram_parameter("bv", [CH, 1], F32, isOutput=False)
    b49 = nc.declare_dram_parameter("b49", [128, NSH], F32, isOutput=False)
    out_d = nc.declare_dram_parameter("out", [CH, NPOS], F32, isOutput=True)

    NCHUNK = 512
    chunks = [(c0, min(NCHUNK, NPAD - c0)) for c0 in range(0, NPAD, NCHUNK)]

    with tile.TileContext(nc) as tc:
        with (
            tc.tile_pool(name="persist", bufs=1) as pp,
            tc.tile_pool(name="work", bufs=2) as wp,
            tc.tile_pool(name="psum", bufs=2, space="PSUM") as psp,
        ):
            # ---- loads (batched into few DMAs to bound per-inst sem waits) ----
            xt_all = pp.tile([128, 4 * NPAD], F32, tag="xall", name="xall")
            wt_all = pp.tile([128, 4 * 3 * CH], F32, tag="wall", name="wall")
            for s0 in range(0, NPAD, 1024):
                sn = min(1024, NPAD - s0)
                nc.sync.dma_start(
                    xt_all[:].rearrange("p (k n) -> p k n", k=4)[:, :, s0:s0 + sn],
                    xp[:].rearrange("(k p) n -> p k n", p=128)[:, :, s0:s0 + sn])
            nc.sync.dma_start(
                wt_all[:].rearrange("p (k n) -> p k n", k=4),
                wT[:].rearrange("(k p) n -> p k n", p=128))
            xt = [xt_all[:].rearrange("p (k n) -> p k n", k=4)[:, kt, :]
                  for kt in range(4)]
            wt = [wt_all[:].rearrange("p (k n) -> p k n", k=4)[:, kt, :]
                  for kt in range(4)]
            bqk_s = pp.tile([128, 1], F32, tag="bqk", name="bqk")
            bv_s = pp.tile([CH, 1], F32, tag="bv", name="bv")
            b49_s = pp.tile([128, NSH], F32, tag="b49", name="b49")
            nc.sync.dma_start(bqk_s[:], bqk[:])
            nc.sync.dma_start(bv_s[:], bv[:])
            nc.sync.dma_start(b49_s[:], b49[:])

            # ---- conv staging (channel-major, padded domain) ----
            qs = pp.tile([CH, NPAD], BF16, tag="qs", name="qs")
            qs32 = pp.tile([CH, NPAD], F32, tag="qs32", name="qs32")
            ks = pp.tile([CH, NPAD], BF16, tag="ks", name="ks")
            vs = pp.tile([CH, NPAD], BF16, tag="vs", name="vs")

            # PE pre-touch of xall: keeps every real Matmult at <=1 sem wait
            # (walrus S3_LW codegen rejects multi-wait matmuls).
            dmy = psp.tile([1, 1], F32, tag="dmy", name="dmy")
            nc.tensor.matmul(dmy[:], lhsT=xt_all[0:1, 0:1],
                             rhs=xt_all[0:1, 0:1], start=True, stop=True)

            for c0, n in chunks:
                ps_qk = psp.tile([128, NCHUNK], F32, tag="psqk", name="psqk")
                ps_v = psp.tile([CH, NCHUNK], F32, tag="psv", name="psv")
                for kt in range(4):
                    nc.tensor.matmul(
                        ps_qk[:, :n], lhsT=wt[kt][:, 0:128],
                        rhs=xt[kt][:, c0:c0 + n],
                        start=(kt == 0), stop=(kt == 3))
                    nc.tensor.matmul(
                        ps_v[:, :n], lhsT=wt[kt][:, 128:192],
                        rhs=xt[kt][:, c0:c0 + n],
                        start=(kt == 0), stop=(kt == 3))
                sl = slice(c0, c0 + n)
                nc.scalar.activation(qs[0:CH, sl], ps_qk[0:CH, :n], AF.Identity,
                                     bias=bqk_s[0:CH, :])
                nc.scalar.activation(qs32[0:CH, sl], ps_qk[0:CH, :n],
                                     AF.Identity, bias=bqk_s[0:CH, :])
                nc.scalar.activation(ks[0:CH, sl], ps_qk[CH:128, :n], AF.Identity,
                                     bias=bqk_s[CH:128, :])
                nc.scalar.activation(vs[0:CH, sl], ps_v[0:CH, :n], AF.Identity,
                                     bias=bv_s[:])

            # ---- remap to 128-partition attention layout (bf16) ----
            qa = pp.tile([128, H0_POS], BF16, tag="qa", name="qa")
            ka = pp.tile([128, KW0], BF16, tag="ka", name="ka")
            va = pp.tile([128, KW0], BF16, tag="va", name="va")
            nc.vector.memset(qa[CH:128, H1_POS:H0_POS], 0.0)
            nc.vector.memset(ka[CH:128, KW1:KW0], 0.0)
            nc.vector.memset(va[CH:128, KW1:KW0], 0.0)

            qs3 = qs[:].rearrange("a (r c) -> a r c", c=WP)
            # central 56x56 of the padded q plane
            nc.sync.dma_start(
                qa[0:CH, :].rearrange("a (x y) -> a x y", y=W),
                qs3[:, 3:3 + H0_ROWS, 3:3 + W])
            nc.sync.dma_start(
                qa[CH:128, 0:H1_POS].rearrange("a (x y) -> a x y", y=W),
                qs3[:, 3 + H0_ROWS:3 + H, 3:3 + W])
            nc.sync.dma_start(ka[0:CH, :], ks[:, 0:KW0])
            nc.sync.dma_start(ka[CH:128, 0:KW1], ks[:, H1_KOFF:NPAD])
            nc.sync.dma_start(va[0:CH, :], vs[:, 0:KW0])
            nc.sync.dma_start(va[CH:128, 0:KW1], vs[:, H1_KOFF:NPAD])

            # odd-element-shifted copies keep every window 4B-aligned so
            # bf16 tensor_tensor stays in 2x mode for odd j shifts
            kao = pp.tile([128, KW0], BF16, tag="kao", name="kao")
            vao = pp.tile([128, KW0], BF16, tag="vao", name="vao")
            nc.scalar.copy(kao[:, 0:KW0 - 1], ka[:, 1:KW0])
            nc.scalar.copy(vao[:, 0:KW0 - 1], va[:, 1:KW0])

            qa32 = pp.tile([128, H0_POS], F32, tag="qa32", name="qa32")
            nc.vector.memset(qa32[CH:128, H1_POS:H0_POS], 0.0)
            qs323 = qs32[:].rearrange("a (r c) -> a r c", c=WP)
            nc.sync.dma_start(
                qa32[0:CH, :].rearrange("a (x y) -> a x y", y=W),
                qs323[:, 3:3 + H0_ROWS, 3:3 + W])
            nc.sync.dma_start(
                qa32[CH:128, 0:H1_POS].rearrange("a (x y) -> a x y", y=W),
                qs323[:, 3 + H0_ROWS:3 + H, 3:3 + W])

            qa3 = qa[:].rearrange("a (x y) -> a x y", y=W)        # [128,32,56]

            def win(t, i, j):
                src_t, jj = (t[0], j) if j % 2 == 0 else (t[1], j - 1)
                t3 = src_t[:].rearrange("a (r c) -> a r c", c=WP)
                return t3[:, i:i + H0_ROWS, jj:jj + W]

            # ---- qk: scores[part, seg, p] (bias added afterwards) ----
            S = pp.tile([128, H0_SEG * NSH], F32, tag="S", name="S")
            S3 = S[:].rearrange("a (s q) -> a s q", q=NSH)
            for p in range(NSH):
                i, j = divmod(p, K)
                prod = wp.tile([128, H0_POS], BF16, tag="prod", name="prod",
                               bufs=2)
                eng = nc.gpsimd if p % 2 == 1 else nc.vector
                eng.tensor_tensor(
                    out=prod[:].rearrange("a (x y) -> a x y", y=W),
                    in0=win((ka, kao), i, j), in1=qa3, op=OP.mult)
                nc.vector.tensor_reduce(
                    out=S3[:, :, p:p + 1],
                    in_=prod[:].rearrange("a (s d) -> a s d", d=SEG),
                    axis=AX.X, op=OP.add)

            # ---- + qsum * bias49 (rank-1), then exp / denominators ----
            qsum = pp.tile([128, H0_SEG], F32, tag="qsum", name="qsum")
            nc.vector.tensor_reduce(
                out=qsum[:],
                in_=qa32[:].rearrange("a (s d) -> a s d", d=SEG),
                axis=AX.X, op=OP.add)
            tb = pp.tile([128, H0_SEG * NSH], F32, tag="tb", name="tb")
            tb3 = tb[:].rearrange("a (s q) -> a s q", q=NSH)
            nc.vector.tensor_tensor(
                out=tb3,
                in0=qsum[:].rearrange("a (s o) -> a s o", o=1).broadcast_to(
                    (128, H0_SEG, NSH)),
                in1=b49_s[:].rearrange("a (o q) -> a o q", o=1).broadcast_to(
                    (128, H0_SEG, NSH)),
                op=OP.mult)
            sb = pp.tile([128, H0_SEG * NSH], F32, tag="sb", name="sb")
            nc.vector.tensor_tensor(out=sb[:], in0=S[:], in1=tb[:], op=OP.add)
            # the rank-1 bias term reaches +-100: must subtract the max
            # before exp or fp32 overflows
            sb3 = sb[:].rearrange("a (s q) -> a s q", q=NSH)
            mx = pp.tile([128, H0_SEG], F32, tag="mx", name="mx")
            nc.vector.tensor_reduce(out=mx[:], in_=sb3, axis=AX.X, op=OP.max)
            nc.vector.tensor_tensor(
                out=sb3, in0=sb3,
                in1=mx[:].rearrange("a (s o) -> a s o", o=1).broadcast_to(
                    (128, H0_SEG, NSH)),
                op=OP.subtract)
            E = pp.tile([128, H0_SEG * NSH], F32, tag="E", name="E")
            nc.scalar.activation(E[:], sb[:], AF.Exp)
            E3 = E[:].rearrange("a (s q) -> a s q", q=NSH)
            den = pp.tile([128, H0_SEG], F32, tag="den", name="den")
            nc.vector.tensor_reduce(out=den[:], in_=E3, axis=AX.X, op=OP.add)
            rcp = pp.tile([128, H0_SEG], F32, tag="rcp", name="rcp")
            nc.vector.reciprocal(rcp[:], den[:])

            # ---- av: acc[part, pos] = sum_p w_p * V_win_p (bf16 chain) ----
            # two independent accumulator chains: DVE owns 37 shifts,
            # GPSIMD owns 12 (p%4==2) end-to-end (mul+add), combined once
            accA = pp.tile([128, H0_POS], BF16, tag="accA", name="accA")
            accB = pp.tile([128, H0_POS], BF16, tag="accB", name="accB")
            accPA = pp.tile([128, H0_POS], BF16, tag="accPA", name="accPA")
            accPB = pp.tile([128, H0_POS], BF16, tag="accPB", name="accPB")
            dve_n = pool_n = 0
            for p in range(NSH):
                i, j = divmod(p, K)
                on_pool = (p % 3 == 2)
                eng = nc.gpsimd if on_pool else nc.vector
                wexp = wp.tile([128, H0_POS], BF16, tag="wexp", name="wexp",
                               bufs=3)
                nc.scalar.copy(
                    out=wexp[:].rearrange("a (s d) -> a s d", d=SEG),
                    in_=E3[:, :, p:p + 1].broadcast_to((128, H0_SEG, SEG)))
                wx = wexp[:].rearrange("a (x y) -> a x y", y=W)
                vwin = win((va, vao), i, j)
                if on_pool:
                    first, pair = pool_n == 0, (accPA, accPB)
                    pool_n += 1
                    k_n = pool_n
                else:
                    first, pair = dve_n == 0, (accA, accB)
                    dve_n += 1
                    k_n = dve_n
                if first:
                    eng.tensor_tensor(
                        out=pair[0][:].rearrange("a (x y) -> a x y", y=W),
                        in0=wx, in1=vwin, op=OP.mult)
                else:
                    tag = "avtP" if on_pool else "avt"
                    tmp = wp.tile([128, H0_POS], BF16, tag=tag, name=tag,
                                  bufs=2)
                    eng.tensor_tensor(
                        out=tmp[:].rearrange("a (x y) -> a x y", y=W),
                        in0=wx, in1=vwin, op=OP.mult)
                    src_t, dst = pair if k_n % 2 == 0 else (pair[1], pair[0])
                    eng.tensor_tensor(
                        out=dst[:], in0=src_t[:], in1=tmp[:], op=OP.add)
            accD = accA if dve_n % 2 == 1 else accB
            accP = accPA if pool_n % 2 == 1 else accPB
            acc = accB if dve_n % 2 == 1 else accA
            nc.vector.tensor_tensor(out=acc[:], in0=accD[:], in1=accP[:],
                                    op=OP.add)

            # ---- normalize (fp32 out) and store ----
            fin = pp.tile([128, H0_POS], F32, tag="fin", name="fin")
            rcpb = rcp[:].rearrange("a (s o) -> a s o", o=1).broadcast_to(
                (128, H0_SEG, SEG))
            nc.vector.tensor_tensor(
                out=fin[:].rearrange("a (s d) -> a s d", d=SEG),
                in0=acc[:].rearrange("a (s d) -> a s d", d=SEG),
                in1=rcpb, op=OP.mult)
            nc.sync.dma_start(out_d[:, 0:H0_POS], fin[0:CH, :])
            nc.sync.dma_start(out_d[:, H0_POS:NPOS], fin[CH:128, 0:H1_POS])
    return nc


import json


def _legalize_waits(bir_bytes):
    """Walrus codegen rejects >1 semaphore wait per instruction; hoist the
    extras onto NoOps (same engine, immediately before) so every
    instruction carries at most one wait."""
    bir = json.loads(bir_bytes)
    ctr = [0]

    def fix_block(instructions):
        out = []
        for ins in instructions:
            si = ins.get("sync_info")
            if si:
                w = si.get("on_wait") or []
                if len(w) > 1:
                    for extra in w[:-1]:
                        ctr[0] += 1
                        out.append({
                            "debug": ins.get("debug", 0),
                            "engine": ins["engine"],
                            "ins": [], "outs": [],
                            "name": f"I-lw{ctr[0]}",
                            "opcode": "NoOp",
                            "sync_info": {"on_wait": [extra],
                                          "on_update": []},
                        })
                    si["on_wait"] = [w[-1]]
            out.append(ins)
        instructions[:] = out

    def walk(o):
        if isinstance(o, dict):
            if "instructions" in o:
                fix_block(o["instructions"])
            for v in o.values():
                walk(v)
        elif isinstance(o, list):
            for v in o:
                walk(v)

    walk(bir)
    return json.dumps(bir).encode()


_NC_CACHE = {}


def kernel(x, q_w, q_b, k_w, k_b, v_w, v_b, h_pos, w_pos):
    x = np.asarray(x, np.float32)
    xp = np.pad(x[0], ((0, 0), (3, 3), (3, 3))).reshape(C, NPAD)
    bias49 = (np.asarray(h_pos, np.float32).sum(0)
              + np.asarray(w_pos, np.float32).sum(0)).reshape(NSH)
    b49bc = np.ascontiguousarray(np.tile(bias49[None, :], (128, 1)))

    in_maps = []
    chan_lists = []
    for r in range(N_CORES):
        chans = np.array([64 * h + 8 * r + t for h in range(8)
                          for t in range(8)])
        chan_lists.append(chans)
        wq = np.asarray(q_w, np.float32)[chans, :]
        wk = np.asarray(k_w, np.float32)[chans, :]
        wv = np.asarray(v_w, np.float32)[chans, :]
        wT = np.ascontiguousarray(
            np.concatenate([wq.T, wk.T, wv.T], axis=1))
        bqk = np.concatenate([np.asarray(q_b, np.float32)[chans],
                              np.asarray(k_b, np.float32)[chans]])
        in_maps.append({
            "xp": xp,
            "wT": wT,
            "bqk": np.ascontiguousarray(bqk[:, None]),
            "bv": np.ascontiguousarray(
                np.asarray(v_b, np.float32)[chans][:, None]),
            "b49": b49bc,
        })

    if "nc" not in _NC_CACHE:
        nc = _build_nc()
        legal = _legalize_waits(nc.to_json_bytes())
        nc.to_json_bytes = lambda: legal
        _NC_CACHE["nc"] = nc
    res = run_bass_kernel_spmd(_NC_CACHE["nc"], in_maps,
                               list(range(N_CORES)))
    _NC_CACHE["last_results"] = res

    out = np.empty((C, NPOS), np.float32)
    for r in range(N_CORES):
        out[chan_lists[r], :] = np.asarray(res.results[r]["out"])
    return out.reshape(1, C, H, W)


if __name__ == "__main__":
    _build_nc()
    print("build OK")


# revision 18
# speedup vs baseline: 1.1408x; 1.0417x over previous
"""Trainium2 Bass kernel for nn_Attention_layer_12249246728743.

Structure of the reference computation (after untangling the C-order
reshape): per channel c of 512, the 3136 raster positions split into 49
segments of 64 consecutive positions; each segment attends over a 7x7
shifted window of its OWN channel plane (depthwise local attention):

  scores[c,s,p=(i,j)] = sum_d q[c,64s+d] * k[c, win(64s+d, i, j)]
                        + (sum_d q[c,64s+d]) * bias49[p]
  w = softmax_p(scores);  out[c,64s+d] = sum_p w[c,s,p] * v[c, win(...)]

with q/k/v = 1x1 convs of x (k, v on the zero-padded 62x62 domain).

Sharding: channel-parallel across 8 cores. Core r owns channels
{64h + 8r + t : h in 0..7, t in 0..7} (64 channels), so every attention
segment is core-local: no halo, no collectives. x is replicated; weight
rows are gathered per core on host.

On-device per core: 1x1 convs on the PE array (contraction over 512 input
channels, 4 K-tiles), bias folded into PSUM->SBUF eviction on the scalar
engine. Attention runs on DVE+GPSIMD+ACT with a 128-partition layout
(channel, image-half): bf16 window products in DVE 2x mode (odd-shifted
K/V copies keep 4B alignment), fp32 segment-reduces, the rank-1
qsum*bias49 term added once (qsum from an fp32 q path - bf16 there costs
1e-1 scores error), max-subtracted exp on ACT, and two independent bf16
output-accumulator chains (DVE 33 shifts, GPSIMD 16) combined once.
"""

import numpy as np

import concourse.bass as bass
import concourse.mybir as mybir
import concourse.tile as tile
from concourse.bass_utils import run_bass_kernel_spmd

F32 = mybir.dt.float32
BF16 = mybir.dt.bfloat16
AX = mybir.AxisListType
OP = mybir.AluOpType
AF = mybir.ActivationFunctionType

N_CORES = 8
C = 512
H = W = 56
HP = WP = 62          # padded spatial
NPOS = H * W          # 3136
NPAD = HP * WP        # 3844
K = 7
NSH = K * K           # 49 shifts
SEG = 64              # positions per attention segment
NSEG = NPOS // SEG    # 49 segments per channel
CH = 64               # channels per core

# image-half split: half0 = out rows 0..31 (28 segs), half1 = rows 32..55 (21 segs)
H0_ROWS, H1_ROWS = 32, 24
H0_POS, H1_POS = H0_ROWS * W, H1_ROWS * W      # 1792, 1344
H0_SEG, H1_SEG = H0_POS // SEG, H1_POS // SEG  # 28, 21
# padded-row ranges needed per half for the 7-row windows
H0_KROWS, H1_KROWS = H0_ROWS + K - 1, H1_ROWS + K - 1   # 38, 30
KW0, KW1 = H0_KROWS * WP, H1_KROWS * WP                 # 2356, 1860
H1_KOFF = 32 * WP                                       # padded row 32 start


def _build_nc():
    nc = bass.Bass()

    xp = nc.declare_dram_parameter("xp", [C, NPAD], F32, isOutput=False)
    wT = nc.declare_dram_parameter("wT", [C, 3 * CH], F32, isOutput=False)
    bqk = nc.declare_dram_parameter("bqk", [2 * CH, 1], F32, isOutput=False)
    bv = nc.declare_dram_parameter("bv", [CH, 1], F32, isOutput=False)
    b49 = nc.declare_dram_parameter("b49", [128, NSH], F32, isOutput=False)
    out_d = nc.declare_dram_parameter("out", [CH, NPOS], F32, isOutput=True)

    NCHUNK = 512
    chunks = [(c0, min(NCHUNK, NPAD - c0)) for c0 in range(0, NPAD, NCHUNK)]

    with tile.TileContext(nc) as tc:
        with (
            tc.tile_pool(name="persist", bufs=1) as pp,
            tc.tile_pool(name="work", bufs=2) as wp,
            tc.tile_pool(name="psum", bufs=2, space="PSUM") as psp,
        ):
            # ---- loads (batched into few DMAs to bound per-inst sem waits) ----
            xt_all = pp.tile([128, 4 * NPAD], F32, tag="xall", name="xall")
            wt_all = pp.tile([128, 4 * 3 * CH], F32, tag="wall", name="wall")
            for s0 in range(0, NPAD, 1024):
                sn = min(1024, NPAD - s0)
                nc.sync.dma_start(
                    xt_all[:].rearrange("p (k n) -> p k n", k=4)[:, :, s0:s0 + sn],
                    xp[:].rearrange("(k p) n -> p k n", p=128)[:, :, s0:s0 + sn])
            nc.sync.dma_start(
                wt_all[:].rearrange("p (k n) -> p k n", k=4),
                wT[:].rearrange("(k p) n -> p k n", p=128))
            xt = [xt_all[:].rearrange("p (k n) -> p k n", k=4)[:, kt, :]
                  for kt in range(4)]
            wt = [wt_all[:].rearrange("p (k n) -> p k n", k=4)[:, kt, :]
                  for kt in range(4)]
            bqk_s = pp.tile([128, 1], F32, tag="bqk", name="bqk")
            bv_s = pp.tile([CH, 1], F32, tag="bv", name="bv")
            b49_s = pp.tile([128, NSH], F32, tag="b49", name="b49")
            nc.sync.dma_start(bqk_s[:], bqk[:])
            nc.sync.dma_start(bv_s[:], bv[:])
            nc.sync.dma_start(b49_s[:], b49[:])

            # ---- conv staging (channel-major, padded domain) ----
            qs = pp.tile([CH, NPAD], BF16, tag="qs", name="qs")
            qs32 = pp.tile([CH, NPAD], F32, tag="qs32", name="qs32")
            ks = pp.tile([CH, NPAD], BF16, tag="ks", name="ks")
            vs = pp.tile([CH, NPAD], BF16, tag="vs", name="vs")

            # PE pre-touch of xall: keeps every real Matmult at <=1 sem wait
            # (walrus S3_LW codegen rejects multi-wait matmuls).
            dmy = psp.tile([1, 1], F32, tag="dmy", name="dmy")
            nc.tensor.matmul(dmy[:], lhsT=xt_all[0:1, 0:1],
                             rhs=xt_all[0:1, 0:1], start=True, stop=True)

            for c0, n in chunks:
                ps_qk = psp.tile([128, NCHUNK], F32, tag="psqk", name="psqk")
                ps_v = psp.tile([CH, NCHUNK], F32, tag="psv", name="psv")
                for kt in range(4):
                    nc.tensor.matmul(
                        ps_qk[:, :n], lhsT=wt[kt][:, 0:128],
                        rhs=xt[kt][:, c0:c0 + n],
                        start=(kt == 0), stop=(kt == 3))
                    nc.tensor.matmul(
                        ps_v[:, :n], lhsT=wt[kt][:, 128:192],
                        rhs=xt[kt][:, c0:c0 + n],
                        start=(kt == 0), stop=(kt == 3))
                sl = slice(c0, c0 + n)
                nc.scalar.activation(qs[0:CH, sl], ps_qk[0:CH, :n], AF.Identity,
                                     bias=bqk_s[0:CH, :])
                nc.scalar.activation(qs32[0:CH, sl], ps_qk[0:CH, :n],
                                     AF.Identity, bias=bqk_s[0:CH, :])
                nc.scalar.activation(ks[0:CH, sl], ps_qk[CH:128, :n], AF.Identity,
                                     bias=bqk_s[CH:128, :])
                nc.scalar.activation(vs[0:CH, sl], ps_v[0:CH, :n], AF.Identity,
                                     bias=bv_s[:])

            # ---- remap to 128-partition attention layout (bf16) ----
            qa = pp.tile([128, H0_POS], BF16, tag="qa", name="qa")
            ka = pp.tile([128, KW0], BF16, tag="ka", name="ka")
            va = pp.tile([128, KW0], BF16, tag="va", name="va")
            nc.vector.memset(qa[CH:128, H1_POS:H0_POS], 0.0)
            nc.vector.memset(ka[CH:128, KW1:KW0], 0.0)
            nc.vector.memset(va[CH:128, KW1:KW0], 0.0)

            qs3 = qs[:].rearrange("a (r c) -> a r c", c=WP)
            # central 56x56 of the padded q plane
            nc.sync.dma_start(
                qa[0:CH, :].rearrange("a (x y) -> a x y", y=W),
                qs3[:, 3:3 + H0_ROWS, 3:3 + W])
            nc.sync.dma_start(
                qa[CH:128, 0:H1_POS].rearrange("a (x y) -> a x y", y=W),
                qs3[:, 3 + H0_ROWS:3 + H, 3:3 + W])
            nc.sync.dma_start(ka[0:CH, :], ks[:, 0:KW0])
            nc.sync.dma_start(ka[CH:128, 0:KW1], ks[:, H1_KOFF:NPAD])
            nc.sync.dma_start(va[0:CH, :], vs[:, 0:KW0])
            nc.sync.dma_start(va[CH:128, 0:KW1], vs[:, H1_KOFF:NPAD])

            # odd-element-shifted copies keep every window 4B-aligned so
            # bf16 tensor_tensor stays in 2x mode for odd j shifts
            kao = pp.tile([128, KW0], BF16, tag="kao", name="kao")
            vao = pp.tile([128, KW0], BF16, tag="vao", name="vao")
            nc.scalar.copy(kao[:, 0:KW0 - 1], ka[:, 1:KW0])
            nc.scalar.copy(vao[:, 0:KW0 - 1], va[:, 1:KW0])

            qa32 = pp.tile([128, H0_POS], F32, tag="qa32", name="qa32")
            nc.vector.memset(qa32[CH:128, H1_POS:H0_POS], 0.0)
            qs323 = qs32[:].rearrange("a (r c) -> a r c", c=WP)
            nc.sync.dma_start(
                qa32[0:CH, :].rearrange("a (x y) -> a x y", y=W),
                qs323[:, 3:3 + H0_ROWS, 3:3 + W])
            nc.sync.dma_start(
                qa32[CH:128, 0:H1_POS].rearrange("a (x y) -> a x y", y=W),
                qs323[:, 3 + H0_ROWS:3 + H, 3:3 + W])

            qa3 = qa[:].rearrange("a (x y) -> a x y", y=W)        # [128,32,56]

            def win(t, i, j):
                src_t, jj = (t[0], j) if j % 2 == 0 else (t[1], j - 1)
                t3 = src_t[:].rearrange("a (r c) -> a r c", c=WP)
                return t3[:, i:i + H0_ROWS, jj:jj + W]

            # ---- qk: scores[part, seg, p] (bias added afterwards) ----
            S = pp.tile([128, H0_SEG * NSH], F32, tag="S", name="S")
            S3 = S[:].rearrange("a (s q) -> a s q", q=NSH)
            for p in range(NSH):
                i, j = divmod(p, K)
                prod = wp.tile([128, H0_POS], BF16, tag="prod", name="prod",
                               bufs=2)
                eng = nc.gpsimd if p % 2 == 1 else nc.vector
                eng.tensor_tensor(
                    out=prod[:].rearrange("a (x y) -> a x y", y=W),
                    in0=win((ka, kao), i, j), in1=qa3, op=OP.mult)
                nc.vector.tensor_reduce(
                    out=S3[:, :, p:p + 1],
                    in_=prod[:].rearrange("a (s d) -> a s d", d=SEG),
                    axis=AX.X, op=OP.add)

            # ---- + qsum * bias49 (rank-1), then exp / denominators ----
            qsum = pp.tile([128, H0_SEG], F32, tag="qsum", name="qsum")
            nc.vector.tensor_reduce(
                out=qsum[:],
                in_=qa32[:].rearrange("a (s d) -> a s d", d=SEG),
                axis=AX.X, op=OP.add)
            tb = pp.tile([128, H0_SEG * NSH], F32, tag="tb", name="tb")
            tb3 = tb[:].rearrange("a (s q) -> a s q", q=NSH)
            nc.vector.tensor_tensor(
                out=tb3,
                in0=qsum[:].rearrange("a (s o) -> a s o", o=1).broadcast_to(
                    (128, H0_SEG, NSH)),
                in1=b49_s[:].rearrange("a (o q) -> a o q", o=1).broadcast_to(
                    (128, H0_SEG, NSH)),
                op=OP.mult)
            sb = pp.tile([128, H0_SEG * NSH], F32, tag="sb", name="sb")
            nc.vector.tensor_tensor(out=sb[:], in0=S[:], in1=tb[:], op=OP.add)
            # the rank-1 bias term reaches +-100: must subtract the max
            # before exp or fp32 overflows
            sb3 = sb[:].rearrange("a (s q) -> a s q", q=NSH)
            mx = pp.tile([128, H0_SEG], F32, tag="mx", name="mx")
            nc.vector.tensor_reduce(out=mx[:], in_=sb3, axis=AX.X, op=OP.max)
            nc.vector.tensor_tensor(
                out=sb3, in0=sb3,
                in1=mx[:].rearrange("a (s o) -> a s o", o=1).broadcast_to(
                    (128, H0_SEG, NSH)),
                op=OP.subtract)
            E = pp.tile([128, H0_SEG * NSH], F32, tag="E", name="E")
            nc.scalar.activation(E[:], sb[:], AF.Exp)
            E3 = E[:].rearrange("a (s q) -> a s q", q=NSH)
            den = pp.tile([128, H0_SEG], F32, tag="den", name="den")
            nc.vector.tensor_reduce(out=den[:], in_=E3, axis=AX.X, op=OP.add)
            rcp = pp.tile([128, H0_SEG], F32, tag="rcp", name="rcp")
            nc.vector.reciprocal(rcp[:], den[:])

            # ---- av: acc[part, pos] = sum_p w_p * V_win_p (bf16 chain) ----
            # two independent accumulator chains: DVE owns 37 shifts,
            # GPSIMD owns 12 (p%4==2) end-to-end (mul+add), combined once
            accA = pp.tile([128, H0_POS], BF16, tag="accA", name="accA")
            accB = pp.tile([128, H0_POS], BF16, tag="accB", name="accB")
            accPA = pp.tile([128, H0_POS], BF16, tag="accPA", name="accPA")
            accPB = pp.tile([128, H0_POS], BF16, tag="accPB", name="accPB")
            dve_n = pool_n = 0
            for p in range(NSH):
                i, j = divmod(p, K)
                on_pool = (p % 3 == 2) or (p % 12 == 1)
                eng = nc.gpsimd if on_pool else nc.vector
                wexp = wp.tile([128, H0_POS], BF16, tag="wexp", name="wexp",
                               bufs=3)
                nc.scalar.copy(
                    out=wexp[:].rearrange("a (s d) -> a s d", d=SEG),
                    in_=E3[:, :, p:p + 1].broadcast_to((128, H0_SEG, SEG)))
                wx = wexp[:].rearrange("a (x y) -> a x y", y=W)
                vwin = win((va, vao), i, j)
                if on_pool:
                    first, pair = pool_n == 0, (accPA, accPB)
                    pool_n += 1
                    k_n = pool_n
                else:
                    first, pair = dve_n == 0, (accA, accB)
                    dve_n += 1
                    k_n = dve_n
                if first:
                    eng.tensor_tensor(
                        out=pair[0][:].rearrange("a (x y) -> a x y", y=W),
                        in0=wx, in1=vwin, op=OP.mult)
                else:
                    tag = "avtP" if on_pool else "avt"
                    tmp = wp.tile([128, H0_POS], BF16, tag=tag, name=tag,
                                  bufs=2)
                    eng.tensor_tensor(
                        out=tmp[:].rearrange("a (x y) -> a x y", y=W),
                        in0=wx, in1=vwin, op=OP.mult)
                    src_t, dst = pair if k_n % 2 == 0 else (pair[1], pair[0])
                    eng.tensor_tensor(
                        out=dst[:], in0=src_t[:], in1=tmp[:], op=OP.add)
            accD = accA if dve_n % 2 == 1 else accB
            accP = accPA if pool_n % 2 == 1 else accPB
            acc = accB if dve_n % 2 == 1 else accA
            nc.vector.tensor_tensor(out=acc[:], in0=accD[:], in1=accP[:],
                                    op=OP.add)

            # ---- normalize (fp32 out) and store ----
            fin = pp.tile([128, H0_POS], F32, tag="fin", name="fin")
            rcpb = rcp[:].rearrange("a (s o) -> a s o", o=1).broadcast_to(
                (128, H0_SEG, SEG))
            nc.vector.tensor_tensor(
                out=fin[:].rearrange("a (s d) -> a s d", d=SEG),
                in0=acc[:].rearrange("a (s d) -> a s d", d=SEG),
                in1=rcpb, op=OP.mult)
            nc.sync.dma_start(out_d[:, 0:H0_POS], fin[0:CH, :])
            nc.sync.dma_start(out_d[:, H0_POS:NPOS], fin[CH:128, 0:H1_POS])
    return nc


import json


def _legalize_waits(bir_bytes):
    """Walrus codegen rejects >1 semaphore wait per instruction; hoist the
    extras onto NoOps (same engine, immediately before) so every
    instruction carries at most one wait."""
    bir = json.loads(bir_bytes)
    ctr = [0]

    def fix_block(instructions):
        out = []
        for ins in instructions:
            si = ins.get("sync_info")
            if si:
                w = si.get("on_wait") or []
                if len(w) > 1:
                    for extra in w[:-1]:
                        ctr[0] += 1
                        out.append({
                            "debug": ins.get("debug", 0),
                            "engine": ins["engine"],
                            "ins": [], "outs": [],
                            "name": f"I-lw{ctr[0]}",
                            "opcode": "NoOp",
                            "sync_info": {"on_wait": [extra],
                                          "on_update": []},
                        })
                    si["on_wait"] = [w[-1]]
            out.append(ins)
        instructions[:] = out

    def walk(o):
        if isinstance(o, dict):
            if "instructions" in o:
                fix_block(o["instructions"])
            for v in o.values():
                walk(v)
        elif isinstance(o, list):
            for v in o:
                walk(v)

    walk(bir)
    return json.dumps(bir).encode()


_NC_CACHE = {}


def kernel(x, q_w, q_b, k_w, k_b, v_w, v_b, h_pos, w_pos):
    x = np.asarray(x, np.float32)
    xp = np.pad(x[0], ((0, 0), (3, 3), (3, 3))).reshape(C, NPAD)
    bias49 = (np.asarray(h_pos, np.float32).sum(0)
              + np.asarray(w_pos, np.float32).sum(0)).reshape(NSH)
    b49bc = np.ascontiguousarray(np.tile(bias49[None, :], (128, 1)))

    in_maps = []
    chan_lists = []
    for r in range(N_CORES):
        chans = np.array([64 * h + 8 * r + t for h in range(8)
                          for t in range(8)])
        chan_lists.append(chans)
        wq = np.asarray(q_w, np.float32)[chans, :]
        wk = np.asarray(k_w, np.float32)[chans, :]
        wv = np.asarray(v_w, np.float32)[chans, :]
        wT = np.ascontiguousarray(
            np.concatenate([wq.T, wk.T, wv.T], axis=1))
        bqk = np.concatenate([np.asarray(q_b, np.float32)[chans],
                              np.asarray(k_b, np.float32)[chans]])
        in_maps.append({
            "xp": xp,
            "wT": wT,
            "bqk": np.ascontiguousarray(bqk[:, None]),
            "bv": np.ascontiguousarray(
                np.asarray(v_b, np.float32)[chans][:, None]),
            "b49": b49bc,
        })

    if "nc" not in _NC_CACHE:
        nc = _build_nc()
        legal = _legalize_waits(nc.to_json_bytes())
        nc.to_json_bytes = lambda: legal
        _NC_CACHE["nc"] = nc
    res = run_bass_kernel_spmd(_NC_CACHE["nc"], in_maps,
                               list(range(N_CORES)))
    _NC_CACHE["last_results"] = res

    out = np.empty((C, NPOS), np.float32)
    for r in range(N_CORES):
        out[chan_lists[r], :] = np.asarray(res.results[r]["out"])
    return out.reshape(1, C, H, W)


if __name__ == "__main__":
    _build_nc()
    print("build OK")
